# revision 1
# baseline (speedup 1.0000x reference)
"""Trainium2 Bass kernel for a 2-layer GCN + linear head (SPMD over 8 cores).

Strategy (per sharding hint): nodes (rows of x / output) are sharded across 8
cores; edges are partitioned by target node. Each layer:
  1. every core computes its shard of the feature table  T = dinv * (h @ W)
     (dinv = rsqrt(weighted in-degree + 1), computed once on device),
  2. AllGather -> full table (bf16) in each core's DRAM,
  3. each core gathers T[src] for its edges (dma_gather over 4 SWDGE queues)
     and scatter-adds into its target windows via S-matrix matmuls on the PE
     (S[e,t] = w_e * (col_e == t), built in one DVE tensor_scalar op),
  4. retire: out = relu(dinv * (msg_sum + T_local) + b)  (self-loop folded in).
Head: out = h2 @ Wout.T + bout computed via PE transpose + matmul.

Edges are bucketed host-side by (target-window of 128 nodes, source-chunk of
25088 rows) so gather indices fit int16; buckets are padded to a static
capacity so the SPMD program is identical on every core.
"""

import os
import sys

sys.path.insert(0, "/opt/trn_rl_repo")

SKIP_MAIN = os.environ.get("KERNEL_SKIP_MAIN") == "1"
SKIP_AG = os.environ.get("KERNEL_SKIP_AG") == "1"
SKIP_DEG = os.environ.get("KERNEL_SKIP_DEG") == "1"
REPS = int(os.environ.get("KERNEL_REPS", "1"))
FAKE_GATHER = os.environ.get("KERNEL_FAKE_GATHER") == "1"

import numpy as np
import ml_dtypes

import concourse.bass as bass
import concourse.mybir as mybir
import concourse.tile as tile
from concourse import bacc, library_config
from concourse.bass_utils import run_bass_kernel_spmd

BF16 = mybir.dt.bfloat16
F32 = mybir.dt.float32
I16 = mybir.dt.int16

NCORES = 8
F = 128  # feature width (IN_F == HID_F == 128)
NPRED = 16


class Cfg:
    def __init__(self, n_nodes, cap):
        self.n = n_nodes
        self.per = n_nodes // NCORES  # nodes per core
        self.nwin = (self.per + 127) // 128  # target windows per core
        self.nwpad = self.nwin * 128
        self.nchunk = 4
        self.chspan = ((n_nodes + self.nchunk * 128 - 1) // (self.nchunk * 128)) * 128
        assert self.chspan <= 32768
        self.cap = cap  # slots per (window, chunk) bucket; multiple of 128
        self.tpb = cap // 128  # tiles per bucket
        # window groups of 2 (PSUM: 2 windows x 4 bufs = 8 banks, 4 groups in flight)
        self.groups = []
        w = 0
        while w < self.nwin:
            g = min(2, self.nwin - w)
            self.groups.append(list(range(w, w + g)))
            w += g
        # slot layout (identical for every core): for g: for c: for w in g: cap
        self.bucket_base = {}
        pos = 0
        for wl in self.groups:
            for c in range(self.nchunk):
                for w in wl:
                    self.bucket_base[(w, c)] = pos
                    pos += cap
        self.nslot = pos
        self.ntile = pos // 128
        self.max_gt = max(len(wl) for wl in self.groups) * self.tpb
        # gather calls: per (g,c) split the len(wl)*tpb tiles into runs of <= 8
        # (<=1024 idxs per call, SWDGE ring limit)
        self.calls = []  # list of (slot_start, n_tiles) in slot order
        for wl in self.groups:
            for c in range(self.nchunk):
                s0 = self.bucket_base[(wl[0], c)]
                t = len(wl) * self.tpb
                off = 0
                while off < t:
                    ct = min(8, t - off)
                    self.calls.append((s0 + off * 128, ct))
                    off += ct


def build_nc(cfg: Cfg):
    nc = bacc.Bacc("TRN2", target_bir_lowering=False, num_swdge_queues=4)
    per, nwin, nwpad = cfg.per, cfg.nwin, cfg.nwpad
    ntile, nslot = cfg.ntile, cfg.nslot

    # inputs (per core)
    xTloc = nc.dram_tensor("xTloc", [128, nwpad], BF16, kind="ExternalInput")
    idx_d = nc.dram_tensor("idx", [128, nslot // 16], I16, kind="ExternalInput")
    colv_d = nc.dram_tensor("colv", [128, ntile], BF16, kind="ExternalInput")
    wv_d = nc.dram_tensor("wv", [128, ntile], BF16, kind="ExternalInput")
    w1_d = nc.dram_tensor("w1", [128, 128], BF16, kind="ExternalInput")
    w2_d = nc.dram_tensor("w2", [128, 128], BF16, kind="ExternalInput")
    wout_d = nc.dram_tensor("woutT", [128, NPRED], BF16, kind="ExternalInput")
    b1_d = nc.dram_tensor("b1bc", [128, 128], F32, kind="ExternalInput")
    b2_d = nc.dram_tensor("b2bc", [128, 128], F32, kind="ExternalInput")
    bout_d = nc.dram_tensor("boutbc", [NPRED, 128], F32, kind="ExternalInput")
    iota_d = nc.dram_tensor("iota", [128, cfg.nchunk * cfg.max_gt * 128], BF16, kind="ExternalInput")
    ident_d = nc.dram_tensor("ident", [128, 128], BF16, kind="ExternalInput")
    ones_d = nc.dram_tensor("ones", [128, 1], BF16, kind="ExternalInput")

    outT = nc.dram_tensor("outT", [NPRED, nwpad], F32, kind="ExternalOutput")

    # internal DRAM
    table1 = nc.dram_tensor("table1", [cfg.n, 128], BF16)
    table2 = nc.dram_tensor("table2", [cfg.n, 128], BF16)
    ag_in = nc.dram_tensor("ag_in", [per, 128], BF16)

    qctr = [0]

    with tile.TileContext(nc) as tc:
        with (
            tc.tile_pool(name="const", bufs=1) as cp,
            tc.tile_pool(name="big", bufs=1) as bigp,
            tc.tile_pool(name="work", bufs=4) as wp,
            tc.tile_pool(name="gat", bufs=12) as gp,
            tc.tile_pool(name="idxp", bufs=4) as idxp,
            tc.tile_pool(name="ret", bufs=6) as rp,
        ):
            nc.gpsimd.load_library(library_config.mlp)
            iota_t = cp.tile([128, cfg.nchunk * cfg.max_gt * 128], BF16)
            nc.sync.dma_start(iota_t[:], iota_d[:])
            ident_t = cp.tile([128, 128], BF16)
            nc.sync.dma_start(ident_t[:], ident_d[:])
            w1_t = cp.tile([128, 128], BF16)
            nc.sync.dma_start(w1_t[:], w1_d[:])
            w2_t = cp.tile([128, 128], BF16)
            nc.sync.dma_start(w2_t[:], w2_d[:])
            wout_t = cp.tile([128, NPRED], BF16)
            nc.sync.dma_start(wout_t[:], wout_d[:])
            b1_t = cp.tile([128, 128], F32)
            nc.sync.dma_start(b1_t[:], b1_d[:])
            b2_t = cp.tile([128, 128], F32)
            nc.sync.dma_start(b2_t[:], b2_d[:])
            bout_t = cp.tile([NPRED, 128], F32)
            nc.sync.dma_start(bout_t[:], bout_d[:])
            ones_t = cp.tile([128, 1], BF16)
            nc.sync.dma_start(ones_t[:], ones_d[:])
            colv_t = cp.tile([128, ntile], BF16)
            nc.sync.dma_start(colv_t[:], colv_d[:])
            wv_t = cp.tile([128, ntile], BF16)
            nc.sync.dma_start(wv_t[:], wv_d[:])
            xTl_t = cp.tile([128, nwpad], BF16)
            nc.sync.dma_start(xTl_t[:], xTloc[:])

            deg_t = cp.tile([128, nwin], F32)
            dinv_t = cp.tile([128, nwin], F32)
            xwloc_t = bigp.tile([128, nwpad], BF16)  # dinv*(h@W) local shard
            h_t = bigp.tile([128, nwpad], BF16)  # layer output [target, feat]
            hT_t = bigp.tile([128, nwpad], BF16)  # transposed layer output
            if SKIP_MAIN:
                nc.vector.memset(h_t[:], 0.0)
            if SKIP_DEG:
                nc.vector.memset(deg_t[:], 0.0)

            import itertools
            _ctr = itertools.count()

            def s_batch(t0, gt):
                sb = wp.tile([128, cfg.nchunk * cfg.max_gt * 128], BF16, tag="Sb", name="sb")
                v3 = sb[:, : gt * 128].rearrange("p (k f) -> p k f", f=128)
                colb = colv_t[:, t0 : t0 + gt].to_broadcast([128, gt, 128])
                wb = wv_t[:, t0 : t0 + gt].to_broadcast([128, gt, 128])
                nc.vector.tensor_tensor(
                    out=v3, in0=iota_t[:, : gt * 128].rearrange("p (k f) -> p k f", f=128),
                    in1=colb, op=mybir.AluOpType.is_equal,
                )
                nc.vector.tensor_tensor(out=v3, in0=v3, in1=wb, op=mybir.AluOpType.mult)
                return sb

            # ---- P1: weighted in-degree -> dinv ----
            for _rep in range(REPS):
              with tc.tile_pool(name=f"psdeg{_rep}", bufs=2, space="PSUM") as pd:
                  for wl in ([] if SKIP_DEG else cfg.groups):
                      ps = {w: pd.tile([128, 1], F32, tag=f"d{i}", name=f"d{i}") for i, w in enumerate(wl)}
                      gt = len(wl) * cfg.tpb
                      sb = s_batch(cfg.bucket_base[(wl[0], 0)] // 128, cfg.nchunk * gt)
                      for c in range(cfg.nchunk):
                          for wi, w in enumerate(wl):
                              for k in range(cfg.tpb):
                                  j = c * gt + wi * cfg.tpb + k
                                  nc.tensor.matmul(
                                      ps[w][:],
                                      lhsT=sb[:, j * 128 : (j + 1) * 128],
                                      rhs=ones_t[:],
                                      start=(c == 0 and k == 0),
                                      stop=(c == cfg.nchunk - 1 and k == cfg.tpb - 1),
                                  )
                      for w in wl:
                          nc.scalar.copy(out=deg_t[:, w : w + 1], in_=ps[w][:])
              # dinv = 1/sqrt(deg + 1)
              tmp = wp.tile([128, nwin], F32, tag="dtmp")
              nc.vector.tensor_scalar_add(out=tmp[:], in0=deg_t[:], scalar1=1.0)
              nc.vector.reciprocal(out=tmp[:], in_=tmp[:])
              nc.scalar.sqrt(out=dinv_t[:], in_=tmp[:])

              # ---- shared: build local shard of table, AG to full table ----
              def build_table(lhsT_full, w_t, table):
                  with tc.tile_pool(name=f"psb{_rep}{id(table)%97}", bufs=3, space="PSUM") as pb:
                      for w in range(nwin):
                          cs = slice(w * 128, (w + 1) * 128)
                          ps = pb.tile([128, 128], F32, tag="pb")
                          nc.tensor.matmul(
                              ps[:], lhsT=lhsT_full[:, cs], rhs=w_t[:], start=True, stop=True
                          )
                          nc.vector.tensor_scalar(
                              out=xwloc_t[:, cs],
                              in0=ps[:],
                              scalar1=dinv_t[:, w : w + 1],
                              scalar2=None,
                              op0=mybir.AluOpType.mult,
                          )
                          hi = min((w + 1) * 128, per)
                          if hi > w * 128:
                              rows = hi - w * 128
                              nc.sync.dma_start(
                                  ag_in[w * 128 : hi, :], xwloc_t[:rows, cs]
                              )
                  if not SKIP_AG:
                      nc.gpsimd.collective_compute(
                          "AllGather",
                          mybir.AluOpType.bypass,
                          replica_groups=[list(range(NCORES))],
                          ins=[ag_in[:, :]],
                          outs=[table[:, :]],
                      )

              # ---- main pass: gather + scatter ----
              def main_pass(table, b_t):
                  call_i = 0
                  with tc.tile_pool(name=f"psm{_rep}{id(table)%97}", bufs=4, space="PSUM") as pm:
                      gpos = 0
                      for wl in cfg.groups:
                          ps = {
                              w: pm.tile([128, 128], F32, tag=f"m{i}", name=f"m{i}")
                              for i, w in enumerate(wl)
                          }
                          gtiles = len(wl) * cfg.tpb
                          idxt = idxp.tile([128, gtiles * cfg.nchunk * 8], I16, tag="idx")
                          nc.sync.dma_start(
                              idxt[:],
                              idx_d[:, gpos // 16 : gpos // 16 + gtiles * cfg.nchunk * 8],
                          )
                          # issue ALL the group's gather calls first so the 4
                          # SWDGE queues stream back-to-back
                          ioff = 0
                          gts = {}
                          for c in range(cfg.nchunk):
                              base = c * cfg.chspan
                              hi = min(base + cfg.chspan, cfg.n)
                              off = 0
                              while off < gtiles:
                                  ct = min(8, gtiles - off)
                                  gt = gp.tile([128, 8, 128], BF16, tag="g")
                                  nidx = ct * 128
                                  nc.gpsimd.dma_gather(
                                      gt[:, :ct, :],
                                      table[base:hi, :],
                                      idxt[:, ioff : ioff + nidx // 16],
                                      nidx,
                                      nidx,
                                      128,
                                      queue_num=qctr[0] % 4,
                                  )
                                  qctr[0] += 1
                                  for j in range(ct):
                                      gts[(c, off + j)] = (gt, j)
                                  off += ct
                                  ioff += nidx // 16
                                  call_i += 1
                          sbt = s_batch(
                              cfg.bucket_base[(wl[0], 0)] // 128,
                              cfg.nchunk * gtiles,
                          )
                          for c in range(cfg.nchunk):
                              for wi, w in enumerate(wl):
                                  for k in range(cfg.tpb):
                                      j = wi * cfg.tpb + k
                                      gt, jj = gts[(c, j)]
                                      jj2 = c * gtiles + j
                                      nc.tensor.matmul(
                                          ps[w][:],
                                          lhsT=sbt[:, jj2 * 128 : (jj2 + 1) * 128],
                                          rhs=gt[:, jj, :],
                                          start=(c == 0 and k == 0),
                                          stop=(c == cfg.nchunk - 1 and k == cfg.tpb - 1),
                                      )
                          gpos += gtiles * cfg.nchunk * 128
                          # retire group
                          for w in wl:
                              cs = slice(w * 128, (w + 1) * 128)
                              t2 = rp.tile([128, 128], F32, tag="t2")
                              nc.vector.tensor_tensor(
                                  out=t2[:],
                                  in0=xwloc_t[:, cs],
                                  in1=ps[w][:],
                                  op=mybir.AluOpType.add,
                              )
                              t4 = rp.tile([128, 128], F32, tag="t4")
                              nc.vector.scalar_tensor_tensor(
                                  out=t4[:],
                                  in0=t2[:],
                                  scalar=dinv_t[:, w : w + 1],
                                  in1=b_t[:],
                                  op0=mybir.AluOpType.mult,
                                  op1=mybir.AluOpType.add,
                              )
                              nc.scalar.activation(
                                  out=h_t[:, cs],
                                  in_=t4[:],
                                  func=mybir.ActivationFunctionType.Relu,
                              )

              # ---- transpose h -> hT ----
              def transpose_h():
                  with tc.tile_pool(name=f"pst{_rep}{next(_ctr)}", bufs=3, space="PSUM") as pt:
                      for w in range(nwin):
                          cs = slice(w * 128, (w + 1) * 128)
                          ps = pt.tile([128, 128], BF16, tag="t")
                          nc.tensor.transpose(ps[:], h_t[:, cs], ident_t[:])
                          nc.scalar.copy(out=hT_t[:, cs], in_=ps[:])

              # ---- layer 1 ----
              build_table(xTl_t, w1_t, table1)
              if not SKIP_MAIN:
                  main_pass(table1, b1_t)
              transpose_h()
              # ---- layer 2 ----
              build_table(hT_t, w2_t, table2)
              if not SKIP_MAIN:
                  main_pass(table2, b2_t)
              transpose_h()
              # ---- head: outT[:, w] = Wout.T^T @ h2T + bout ----
              with tc.tile_pool(name=f"psh{_rep}", bufs=3, space="PSUM") as ph:
                  for w in range(nwin):
                      cs = slice(w * 128, (w + 1) * 128)
                      ps = ph.tile([NPRED, 128], F32, tag="h")
                      nc.tensor.matmul(
                          ps[:], lhsT=wout_t[:], rhs=hT_t[:, cs], start=True, stop=True
                      )
                      ot = rp.tile([NPRED, 128], F32, tag="ot")
                      nc.vector.tensor_tensor(
                          out=ot[:], in0=ps[:], in1=bout_t[:], op=mybir.AluOpType.add
                      )
                      nc.sync.dma_start(outT[:, cs], ot[:])
    nc.compile()
    return nc


def prep_inputs(cfg: Cfg, x, edge_index, edge_weight, W1, b1, W2, b2, Wout, bout):
    per, nwin, nwpad, cap = cfg.per, cfg.nwin, cfg.nwpad, cfg.cap
    row = np.asarray(edge_index[0], dtype=np.int64)
    col = np.asarray(edge_index[1], dtype=np.int64)
    wgt = np.asarray(edge_weight, dtype=np.float32)

    core = col // per
    col_local = col - core * per
    win = col_local >> 7
    cw = (col_local & 127).astype(np.float32)
    chunk = row // cfg.chspan
    idx16 = (row - chunk * cfg.chspan).astype(np.int16)

    nbuck_core = nwin * cfg.nchunk
    bid = (core * nbuck_core + win * cfg.nchunk + chunk).astype(np.int64)
    order = np.argsort(bid, kind="stable")
    bid_s = bid[order]
    counts = np.bincount(bid_s, minlength=NCORES * nbuck_core)
    assert counts.max() <= cap, f"bucket overflow: {counts.max()} > {cap}"
    starts = np.zeros(NCORES * nbuck_core + 1, dtype=np.int64)
    np.cumsum(counts, out=starts[1:])
    rank = np.arange(len(order)) - starts[bid_s]

    base_1core = np.zeros(nbuck_core, dtype=np.int64)
    for (w, c), b in cfg.bucket_base.items():
        base_1core[w * cfg.nchunk + c] = b
    slot = base_1core[bid_s % nbuck_core] + rank
    core_s = bid_s // nbuck_core

    idx_all = np.zeros((NCORES, cfg.nslot), dtype=np.int16)
    colv = np.zeros((NCORES, cfg.nslot), dtype=np.float32)
    wv = np.zeros((NCORES, cfg.nslot), dtype=np.float32)
    idx_all[core_s, slot] = idx16[order]
    colv[core_s, slot] = cw[order]
    wv[core_s, slot] = wgt[order]

    # wrap idx per call: [128, nslot/16]
    idx_wrapped = np.zeros((NCORES, 128, cfg.nslot // 16), dtype=np.int16)
    for s0, nt in cfg.calls:
        n = nt * 128
        blk = idx_all[:, s0 : s0 + n].reshape(NCORES, n // 16, 16)
        blk = np.transpose(blk, (0, 2, 1))  # [NCORES, 16, n/16]
        idx_wrapped[:, :, s0 // 16 : (s0 + n) // 16] = np.tile(blk, (1, 8, 1))

    # metadata tiles: value for slot s at [s%128, s//128]
    colv_t = np.transpose(colv.reshape(NCORES, cfg.ntile, 128), (0, 2, 1)).astype(
        ml_dtypes.bfloat16
    )
    wv_t = np.transpose(wv.reshape(NCORES, cfg.ntile, 128), (0, 2, 1)).astype(
        ml_dtypes.bfloat16
    )

    xpad = np.zeros((NCORES, nwpad, 128), dtype=np.float32)
    xv = np.asarray(x, dtype=np.float32)
    for d in range(NCORES):
        xpad[d, :per] = xv[d * per : (d + 1) * per]
    xTloc = np.ascontiguousarray(np.transpose(xpad, (0, 2, 1))).astype(ml_dtypes.bfloat16)

    iota = np.broadcast_to(
        np.tile(np.arange(128, dtype=np.float32), cfg.nchunk * cfg.max_gt),
        (128, cfg.nchunk * cfg.max_gt * 128),
    ).astype(ml_dtypes.bfloat16)
    ident = np.eye(128, dtype=np.float32).astype(ml_dtypes.bfloat16)
    common = {
        "colv": None,
        "w1": np.asarray(W1, np.float32).astype(ml_dtypes.bfloat16),
        "w2": np.asarray(W2, np.float32).astype(ml_dtypes.bfloat16),
        "woutT": np.ascontiguousarray(np.asarray(Wout, np.float32).T).astype(
            ml_dtypes.bfloat16
        ),
        "b1bc": np.broadcast_to(np.asarray(b1, np.float32), (128, 128)).copy(),
        "b2bc": np.broadcast_to(np.asarray(b2, np.float32), (128, 128)).copy(),
        "boutbc": np.broadcast_to(
            np.asarray(bout, np.float32)[:, None], (NPRED, 128)
        ).copy(),
        "iota": iota,
        "ident": ident,
        "ones": np.ones((128, 1), np.float32).astype(ml_dtypes.bfloat16),
    }
    in_maps = []
    for d in range(NCORES):
        m = dict(common)
        m["colv"] = colv_t[d]
        m["wv"] = wv_t[d]
        m["idx"] = idx_wrapped[d]
        m["xTloc"] = xTloc[d]
        in_maps.append(m)
    return in_maps


_CACHE = {}


def run(cfg, x, edge_index, edge_weight, W1, b1, W2, b2, Wout, bout):
    in_maps = prep_inputs(cfg, x, edge_index, edge_weight, W1, b1, W2, b2, Wout, bout)
    key = (cfg.n, cfg.cap)
    if key not in _CACHE:
        _CACHE[key] = build_nc(cfg)
    nc = _CACHE[key]
    res = run_bass_kernel_spmd(nc, in_maps, list(range(NCORES)))
    outs = []
    for d in range(NCORES):
        ot = res.results[d]["outT"]  # [16, nwpad]
        outs.append(ot[:, : cfg.per].T)
    return np.ascontiguousarray(np.concatenate(outs, axis=0), dtype=np.float32)


def kernel(x, edge_index, edge_weight, W1, b1, W2, b2, Wout, bout):
    cfg = Cfg(100000, 640)
    return run(cfg, x, edge_index, edge_weight, W1, b1, W2, b2, Wout, bout)


if __name__ == "__main__":
    # small smoke test: N=4096 nodes, 65536 edges
    rng = np.random.default_rng(0)
    n, e = 4096, 65536
    x = rng.standard_normal((n, 128)).astype(np.float32)
    ei = rng.integers(0, n, (2, e)).astype(np.int64)
    ew = rng.random(e).astype(np.float32)
    W1 = (rng.standard_normal((128, 128)) / np.sqrt(128)).astype(np.float32)
    W2 = (rng.standard_normal((128, 128)) / np.sqrt(128)).astype(np.float32)
    Wout = (rng.standard_normal((16, 128)) / np.sqrt(128)).astype(np.float32)
    b1 = np.zeros(128, np.float32)
    b2 = np.zeros(128, np.float32)
    bout = np.zeros(16, np.float32)

    # numpy reference (f32)
    def gcn(xx, W, b):
        deg = np.bincount(ei[1], weights=ew, minlength=n) + 1.0
        dinv = 1.0 / np.sqrt(deg)
        xw = xx @ W
        msg = xw[ei[0]] * (dinv[ei[0]] * ew * dinv[ei[1]])[:, None]
        out = np.zeros_like(xw)
        np.add.at(out, ei[1], msg)
        out += xw * (dinv**2)[:, None]
        return np.maximum(out + b, 0.0)

    h = gcn(x, W1, b1)
    h = gcn(h, W2, b2)
    ref = h @ Wout.T + bout

    # small cfg: per-core 512 nodes, 4 windows; bucket mean = 65536/8/4/4=512... cap:
    cfg = Cfg(n, 768)
    got = run(cfg, x, ei, ew, W1, b1, W2, b2, Wout, bout)
    err = np.abs(got - ref).max() / (np.abs(ref).max() + 1e-9)
    l2 = np.linalg.norm(got - ref) / np.linalg.norm(ref)
    print(f"SMOKE: max rel err {err:.3e}   l2 rel {l2:.3e}")



# revision 5
# speedup vs baseline: 17.6094x; 17.6094x over previous
"""Trainium2 Bass kernel v2 for 2-layer GCN + linear head (SPMD over 8 cores).

Key differences vs v1:
  - Aggregation commutes with the weight matmul: agg = sum_e norm_e * x[src]
    is computed on RAW features, then h = relu(aggT^T @ W + b). Layer 1 needs
    no AllGather at all: every core gets a replicated bf16 copy of x (host
    input) and gathers straight from it.
  - deg/dinv/norm are host-precomputed; norm_e folded into the S-matrix
    weights, self-loop handled via dsq = dinv^2 input. No device degree pass.
  - Bigger SWDGE gather calls: 6-window groups x 1 chunk = 30 tiles (3840
    idxs) per dma_gather, with dynamic_dma_scratch_size=65536 (4096-desc
    rings x 4 queues). 68 calls/layer instead of 392.
  - PSUM: 6 accumulator banks (one per window in group) + 2-bank transform
    pool (PE transpose + W-matmul + head).
  - ACT engine used for Relu only (no function-table thrash); copies on DVE.
  - AllGather #2 (h1 table) output to addr_space="Shared" DRAM (KERNEL_AG=local
    to fall back).
"""

import os
import sys

sys.path.insert(0, "/opt/trn_rl_repo")

SKIP_AG = os.environ.get("KERNEL_SKIP_AG") == "1"
SKIP_MAIN = os.environ.get("KERNEL_SKIP_MAIN") == "1"
GONLY = os.environ.get("KERNEL_GONLY") == "1"  # gathers only, no compute
NOGATHER = os.environ.get("KERNEL_NOGATHER") == "1"  # compute on const tile
SFUSED = os.environ.get("KERNEL_SFUSED", "0") == "1"  # per-tile fused S build
REPS = int(os.environ.get("KERNEL_REPS", "1"))
AG_MODE = os.environ.get("KERNEL_AG", "shared")
NQUEUES = int(os.environ.get("KERNEL_NQUEUES", "4"))
SCRATCH = int(os.environ.get("KERNEL_SCRATCH", "16384"))
MAXCT = int(os.environ.get("KERNEL_MAXCT", "8"))  # tiles per gather sub-call

import numpy as np
import ml_dtypes

import concourse.bass as bass
import concourse.mybir as mybir
import concourse.tile as tile
from concourse import bacc, library_config
from concourse.bass_utils import run_bass_kernel_spmd

BF16 = mybir.dt.bfloat16
F32 = mybir.dt.float32
I16 = mybir.dt.int16

NCORES = 8
F = 128
NPRED = 16


class Cfg:
    def __init__(self, n_nodes, cap, G=6):
        self.n = n_nodes
        self.per = n_nodes // NCORES
        self.nwin = (self.per + 127) // 128
        self.nwpad = self.nwin * 128
        self.nchunk = 4
        self.chspan = ((n_nodes + self.nchunk * 128 - 1) // (self.nchunk * 128)) * 128
        assert self.chspan <= 32768
        self.cap = cap
        self.tpb = cap // 128
        self.G = min(G, self.nwin)
        self.groups = []
        w = 0
        while w < self.nwin:
            g = min(self.G, self.nwin - w)
            self.groups.append(list(range(w, w + g)))
            w += g
        # slot layout: for g: for c: for w in wl: cap
        self.bucket_base = {}
        pos = 0
        for wl in self.groups:
            for c in range(self.nchunk):
                for w in wl:
                    self.bucket_base[(w, c)] = pos
                    pos += cap
        self.nslot = pos
        self.ntile = pos // 128
        self.max_gt = self.G * self.tpb  # tiles per (group, chunk)
        # sub-calls: the HW SWDGE ring holds 1024 descriptors per queue, so a
        # single dma_gather is capped at MAXCT tiles (MAXCT*128 idxs).
        self.calls = []  # (slot0, ntiles) in kernel issue order
        for wl in self.groups:
            for c in range(self.nchunk):
                s0 = self.bucket_base[(wl[0], c)]
                t = len(wl) * self.tpb
                off = 0
                while off < t:
                    ct = min(MAXCT, t - off)
                    self.calls.append((s0 + off * 128, ct))
                    off += ct


def build_nc(cfg: Cfg):
    nc = bacc.Bacc(
        "TRN2",
        target_bir_lowering=False,
        num_swdge_queues=NQUEUES,
        dynamic_dma_scratch_size=SCRATCH,
    )
    per, nwin, nwpad, tpb = cfg.per, cfg.nwin, cfg.nwpad, cfg.tpb
    ntile, nslot = cfg.ntile, cfg.nslot

    xfull_d = nc.dram_tensor("xfull", [cfg.n, F], BF16, kind="ExternalInput")
    xloc_d = nc.dram_tensor("xloc", [128, nwpad], BF16, kind="ExternalInput")
    dsq_d = nc.dram_tensor("dsq", [128, nwin], F32, kind="ExternalInput")
    idx_d = nc.dram_tensor("idx", [128, nslot // 16], I16, kind="ExternalInput")
    colv_d = nc.dram_tensor("colv", [128, ntile], BF16, kind="ExternalInput")
    wv_d = nc.dram_tensor("wv", [128, ntile], BF16, kind="ExternalInput")
    w1_d = nc.dram_tensor("w1", [128, 128], BF16, kind="ExternalInput")
    w2_d = nc.dram_tensor("w2", [128, 128], BF16, kind="ExternalInput")
    wout_d = nc.dram_tensor("woutT", [128, NPRED], BF16, kind="ExternalInput")
    b1_d = nc.dram_tensor("b1bc", [128, 128], F32, kind="ExternalInput")
    b2_d = nc.dram_tensor("b2col", [128, 1], F32, kind="ExternalInput")
    bout_d = nc.dram_tensor("boutbc", [NPRED, 128], F32, kind="ExternalInput")
    iota_d = nc.dram_tensor("iota", [128, cfg.max_gt * 128], BF16, kind="ExternalInput")
    ident_d = nc.dram_tensor("ident", [128, 128], BF16, kind="ExternalInput")

    outT = nc.dram_tensor("outT", [NPRED, nwpad], F32, kind="ExternalOutput")

    ag_in = nc.dram_tensor("ag_in", [per, F], BF16)
    if AG_MODE == "shared":
        table2 = nc.dram_tensor("table2", [cfg.n, F], BF16, addr_space="Shared")
    else:
        table2 = nc.dram_tensor("table2", [cfg.n, F], BF16)

    qctr = [0]

    with tile.TileContext(nc) as tc:
        with (
            tc.tile_pool(name="const", bufs=1) as cp,
            tc.tile_pool(name="big", bufs=1) as bigp,
            tc.tile_pool(name="gat", bufs=6) as gp,
            tc.tile_pool(name="sbld", bufs=6) as wp,
            tc.tile_pool(name="idxp", bufs=2) as ip,
            tc.tile_pool(name="ret", bufs=4) as rp,
        ):
            nc.gpsimd.load_library(library_config.mlp)
            iota_t = cp.tile([128, cfg.max_gt * 128], BF16)
            nc.sync.dma_start(iota_t[:], iota_d[:])
            ident_t = cp.tile([128, 128], BF16)
            nc.sync.dma_start(ident_t[:], ident_d[:])
            w1_t = cp.tile([128, 128], BF16)
            nc.sync.dma_start(w1_t[:], w1_d[:])
            w2_t = cp.tile([128, 128], BF16)
            nc.sync.dma_start(w2_t[:], w2_d[:])
            wout_t = cp.tile([128, NPRED], BF16)
            nc.sync.dma_start(wout_t[:], wout_d[:])
            b1_t = cp.tile([128, 128], F32)
            nc.sync.dma_start(b1_t[:], b1_d[:])
            b2_t = cp.tile([128, 1], F32)
            nc.sync.dma_start(b2_t[:], b2_d[:])
            bout_t = cp.tile([NPRED, 128], F32)
            nc.sync.dma_start(bout_t[:], bout_d[:])
            colv_t = cp.tile([128, ntile], BF16)
            nc.sync.dma_start(colv_t[:], colv_d[:])
            wv_t = cp.tile([128, ntile], BF16)
            nc.sync.dma_start(wv_t[:], wv_d[:])
            xloc_t = cp.tile([128, nwpad], BF16)
            nc.sync.dma_start(xloc_t[:], xloc_d[:])
            dsq_t = cp.tile([128, nwin], F32)
            nc.sync.dma_start(dsq_t[:], dsq_d[:])

            h_t = bigp.tile([128, nwpad], BF16)  # layer-1 output [target, feat]
            gtc = None
            if NOGATHER:
                gtc = cp.tile([128, cfg.max_gt, 128], BF16)
                nc.vector.memset(gtc[:], 0.0)

            for _rep in range(REPS):
                with (
                    tc.tile_pool(name=f"pacc{_rep}", bufs=1, space="PSUM") as pacc,
                    tc.tile_pool(name=f"ptr{_rep}", bufs=2, space="PSUM") as ptr,
                ):

                    def layer(l, table, selfloop_t):
                        gpos = 0
                        for wl in cfg.groups:
                            gtiles = len(wl) * tpb
                            nidx_c = gtiles * 128  # idxs per (group, chunk) call
                            idxt = ip.tile(
                                [128, cfg.max_gt * cfg.nchunk * 8], I16, tag="idx",
                                name="idxt",
                            )
                            nc.sync.dma_start(
                                idxt[:, : gtiles * cfg.nchunk * 8],
                                idx_d[:, gpos // 16 : gpos // 16 + gtiles * cfg.nchunk * 8],
                            )
                            ps = (
                                {}
                                if GONLY
                                else {
                                    w: pacc.tile(
                                        [128, 128], F32, tag=f"a{i}", name=f"a{i}"
                                    )
                                    for i, w in enumerate(wl)
                                }
                            )
                            for c in range(cfg.nchunk):
                                base = c * cfg.chspan
                                hi = min(base + cfg.chspan, cfg.n)
                                if NOGATHER:
                                    gt = gtc
                                else:
                                    gt = gp.tile(
                                        [128, cfg.max_gt, 128], BF16, tag="g", name="gt"
                                    )
                                    off = 0
                                    while off < gtiles:
                                        ct = min(MAXCT, gtiles - off)
                                        i16_0 = (c * gtiles + off) * 8
                                        nc.gpsimd.dma_gather(
                                            gt[:, off : off + ct, :],
                                            table[base:hi, :],
                                            idxt[:, i16_0 : i16_0 + ct * 8],
                                            ct * 128,
                                            ct * 128,
                                            128,
                                            queue_num=qctr[0] % NQUEUES,
                                        )
                                        qctr[0] += 1
                                        off += ct
                                if GONLY:
                                    continue
                                t0 = cfg.bucket_base[(wl[0], c)] // 128
                                sb = wp.tile([128, cfg.max_gt * 128], BF16, tag="sb", name="sb")
                                if SFUSED:
                                    for k in range(gtiles):
                                        nc.vector.tensor_scalar(
                                            out=sb[:, k * 128 : (k + 1) * 128],
                                            in0=iota_t[:, :128],
                                            scalar1=colv_t[:, t0 + k : t0 + k + 1],
                                            scalar2=wv_t[:, t0 + k : t0 + k + 1],
                                            op0=mybir.AluOpType.is_equal,
                                            op1=mybir.AluOpType.mult,
                                        )
                                else:
                                    v3 = sb[:, : gtiles * 128].rearrange(
                                        "p (k f) -> p k f", f=128
                                    )
                                    nc.vector.tensor_tensor(
                                        out=v3,
                                        in0=iota_t[:, : gtiles * 128].rearrange(
                                            "p (k f) -> p k f", f=128
                                        ),
                                        in1=colv_t[:, t0 : t0 + gtiles].to_broadcast(
                                            [128, gtiles, 128]
                                        ),
                                        op=mybir.AluOpType.is_equal,
                                    )
                                    nc.vector.tensor_tensor(
                                        out=v3,
                                        in0=v3,
                                        in1=wv_t[:, t0 : t0 + gtiles].to_broadcast(
                                            [128, gtiles, 128]
                                        ),
                                        op=mybir.AluOpType.mult,
                                    )
                                for wi, w in enumerate(wl):
                                    for k in range(tpb):
                                        j = wi * tpb + k
                                        nc.tensor.matmul(
                                            ps[w][:],
                                            lhsT=sb[:, j * 128 : (j + 1) * 128],
                                            rhs=gt[:, j, :],
                                            start=(c == 0 and k == 0),
                                            stop=(c == cfg.nchunk - 1 and k == tpb - 1),
                                        )
                            gpos += gtiles * cfg.nchunk * 128
                            # retire + transform per window
                            for wi, w in enumerate([] if GONLY else wl):
                                cs = slice(w * 128, (w + 1) * 128)
                                agg = rp.tile([128, 128], BF16, tag="agg", name="agg")
                                nc.vector.scalar_tensor_tensor(
                                    out=agg[:],
                                    in0=selfloop_t[:, cs],
                                    scalar=dsq_t[:, w : w + 1],
                                    in1=ps[w][:],
                                    op0=mybir.AluOpType.mult,
                                    op1=mybir.AluOpType.add,
                                )
                                pst = ptr.tile([128, 128], BF16, tag="t", name="pst")
                                nc.tensor.transpose(pst[:], agg[:], ident_t[:])
                                aggT = rp.tile([128, 128], BF16, tag="aggT", name="aggT")
                                nc.vector.tensor_scalar_add(aggT[:], pst[:], 0.0)
                                if l == 1:
                                    ps2 = ptr.tile([128, 128], F32, tag="t", name="ps2")
                                    nc.tensor.matmul(
                                        ps2[:], lhsT=aggT[:], rhs=w1_t[:],
                                        start=True, stop=True,
                                    )
                                    t4 = rp.tile([128, 128], F32, tag="t4", name="t4")
                                    nc.vector.tensor_tensor(
                                        out=t4[:], in0=ps2[:], in1=b1_t[:],
                                        op=mybir.AluOpType.add,
                                    )
                                    nc.scalar.activation(
                                        out=h_t[:, cs], in_=t4[:],
                                        func=mybir.ActivationFunctionType.Relu,
                                    )
                                    hi = min((w + 1) * 128, per)
                                    if hi > w * 128:
                                        nc.sync.dma_start(
                                            ag_in[w * 128 : hi, :],
                                            h_t[: hi - w * 128, cs],
                                        )
                                else:
                                    ps2 = ptr.tile([128, 128], F32, tag="t", name="ps2b")
                                    nc.tensor.matmul(
                                        ps2[:], lhsT=w2_t[:], rhs=aggT[:],
                                        start=True, stop=True,
                                    )
                                    h2T = rp.tile([128, 128], BF16, tag="h2T", name="h2T")
                                    nc.vector.tensor_scalar(
                                        out=h2T[:], in0=ps2[:],
                                        scalar1=b2_t[:, 0:1], scalar2=0.0,
                                        op0=mybir.AluOpType.add,
                                        op1=mybir.AluOpType.max,
                                    )
                                    ps3 = ptr.tile([NPRED, 128], F32, tag="t", name="ps3")
                                    nc.tensor.matmul(
                                        ps3[:], lhsT=wout_t[:], rhs=h2T[:],
                                        start=True, stop=True,
                                    )
                                    ot = rp.tile([NPRED, 128], F32, tag="ot", name="ot")
                                    nc.vector.tensor_tensor(
                                        out=ot[:], in0=ps3[:], in1=bout_t[:],
                                        op=mybir.AluOpType.add,
                                    )
                                    nc.sync.dma_start(outT[:, cs], ot[:])

                    layer(1, xfull_d, xloc_t)
                    if not SKIP_AG and not GONLY:
                        nc.gpsimd.collective_compute(
                            "AllGather",
                            mybir.AluOpType.bypass,
                            replica_groups=[list(range(NCORES))],
                            ins=[ag_in[:, :]],
                            outs=[table2[:, :]],
                        )
                    layer(2, table2, h_t)
    nc.compile()
    return nc


def prep_inputs(cfg: Cfg, x, edge_index, edge_weight, W1, b1, W2, b2, Wout, bout):
    per, nwin, nwpad, cap = cfg.per, cfg.nwin, cfg.nwpad, cfg.cap
    n = cfg.n
    row = np.asarray(edge_index[0], dtype=np.int64)
    col = np.asarray(edge_index[1], dtype=np.int64)
    wgt = np.asarray(edge_weight, dtype=np.float32)

    deg = np.bincount(col, weights=wgt, minlength=n).astype(np.float32) + 1.0
    dinv = 1.0 / np.sqrt(deg)
    norm = dinv[row] * wgt * dinv[col]
    dsq = dinv * dinv

    core = col // per
    col_local = col - core * per
    win = col_local >> 7
    cw = (col_local & 127).astype(np.float32)
    chunk = row // cfg.chspan
    idx16 = (row - chunk * cfg.chspan).astype(np.int16)

    nbuck_core = nwin * cfg.nchunk
    bid = (core * nbuck_core + win * cfg.nchunk + chunk).astype(np.int64)
    order = np.argsort(bid, kind="stable")
    bid_s = bid[order]
    counts = np.bincount(bid_s, minlength=NCORES * nbuck_core)
    assert counts.max() <= cap, f"bucket overflow: {counts.max()} > {cap}"
    starts = np.zeros(NCORES * nbuck_core + 1, dtype=np.int64)
    np.cumsum(counts, out=starts[1:])
    rank = np.arange(len(order)) - starts[bid_s]

    base_1core = np.zeros(nbuck_core, dtype=np.int64)
    for (w, c), b in cfg.bucket_base.items():
        base_1core[w * cfg.nchunk + c] = b
    slot = base_1core[bid_s % nbuck_core] + rank
    core_s = bid_s // nbuck_core

    idx_all = np.zeros((NCORES, cfg.nslot), dtype=np.int16)
    colv = np.zeros((NCORES, cfg.nslot), dtype=np.float32)
    wv = np.zeros((NCORES, cfg.nslot), dtype=np.float32)
    idx_all[core_s, slot] = idx16[order]
    colv[core_s, slot] = cw[order]
    wv[core_s, slot] = norm[order]

    # wrap idx per gather sub-call
    idx_wrapped = np.zeros((NCORES, 128, cfg.nslot // 16), dtype=np.int16)
    for s0, nt in cfg.calls:
        nidx = nt * 128
        blk = idx_all[:, s0 : s0 + nidx].reshape(NCORES, nidx // 16, 16)
        blk = np.transpose(blk, (0, 2, 1))
        idx_wrapped[:, :, s0 // 16 : (s0 + nidx) // 16] = np.tile(blk, (1, 8, 1))

    colv_t = np.transpose(colv.reshape(NCORES, cfg.ntile, 128), (0, 2, 1)).astype(
        ml_dtypes.bfloat16
    )
    wv_t = np.transpose(wv.reshape(NCORES, cfg.ntile, 128), (0, 2, 1)).astype(
        ml_dtypes.bfloat16
    )

    xv = np.asarray(x, dtype=np.float32)
    xfull = xv.astype(ml_dtypes.bfloat16)

    xpad = np.zeros((NCORES, nwpad, 128), dtype=np.float32)
    dsqpad = np.zeros((NCORES, nwpad), dtype=np.float32)
    for d in range(NCORES):
        xpad[d, :per] = xv[d * per : (d + 1) * per]
        dsqpad[d, :per] = dsq[d * per : (d + 1) * per]
    # [t, f] per window: [128, nwin*128]
    xloc = (
        np.transpose(xpad.reshape(NCORES, nwin, 128, 128), (0, 2, 1, 3))
        .reshape(NCORES, 128, nwpad)
        .astype(ml_dtypes.bfloat16)
    )
    dsq_t = np.transpose(dsqpad.reshape(NCORES, nwin, 128), (0, 2, 1)).copy()

    iota = np.broadcast_to(
        np.tile(np.arange(128, dtype=np.float32), cfg.max_gt),
        (128, cfg.max_gt * 128),
    ).astype(ml_dtypes.bfloat16)
    ident = np.eye(128, dtype=np.float32).astype(ml_dtypes.bfloat16)

    common = {
        "xfull": xfull,
        "w1": np.asarray(W1, np.float32).astype(ml_dtypes.bfloat16),
        "w2": np.asarray(W2, np.float32).astype(ml_dtypes.bfloat16),
        "woutT": np.ascontiguousarray(np.asarray(Wout, np.float32).T).astype(
            ml_dtypes.bfloat16
        ),
        "b1bc": np.broadcast_to(np.asarray(b1, np.float32), (128, 128)).copy(),
        "b2col": np.asarray(b2, np.float32)[:, None].copy(),
        "boutbc": np.broadcast_to(
            np.asarray(bout, np.float32)[:, None], (NPRED, 128)
        ).copy(),
        "iota": iota,
        "ident": ident,
    }
    in_maps = []
    for d in range(NCORES):
        m = dict(common)
        m["colv"] = colv_t[d]
        m["wv"] = wv_t[d]
        m["idx"] = idx_wrapped[d]
        m["xloc"] = xloc[d]
        m["dsq"] = dsq_t[d]
        in_maps.append(m)
    return in_maps


_CACHE = {}


def run(cfg, x, edge_index, edge_weight, W1, b1, W2, b2, Wout, bout):
    in_maps = prep_inputs(cfg, x, edge_index, edge_weight, W1, b1, W2, b2, Wout, bout)
    key = (cfg.n, cfg.cap, cfg.G, REPS, AG_MODE, SFUSED, GONLY, NOGATHER, SKIP_AG)
    if key not in _CACHE:
        _CACHE[key] = build_nc(cfg)
    nc = _CACHE[key]
    res = run_bass_kernel_spmd(nc, in_maps, list(range(NCORES)))
    outs = []
    for d in range(NCORES):
        ot = res.results[d]["outT"]
        outs.append(ot[:, : cfg.per].T)
    return np.ascontiguousarray(np.concatenate(outs, axis=0), dtype=np.float32)


DEFAULT_G = 6


def kernel(x, edge_index, edge_weight, W1, b1, W2, b2, Wout, bout):
    cfg = Cfg(100000, 640, G=DEFAULT_G)
    return run(cfg, x, edge_index, edge_weight, W1, b1, W2, b2, Wout, bout)


if __name__ == "__main__":
    rng = np.random.default_rng(0)
    n, e = 4096, 65536
    x = rng.standard_normal((n, 128)).astype(np.float32)
    ei = rng.integers(0, n, (2, e)).astype(np.int64)
    ew = rng.random(e).astype(np.float32)
    W1 = (rng.standard_normal((128, 128)) / np.sqrt(128)).astype(np.float32)
    W2 = (rng.standard_normal((128, 128)) / np.sqrt(128)).astype(np.float32)
    Wout = (rng.standard_normal((16, 128)) / np.sqrt(128)).astype(np.float32)
    b1 = rng.standard_normal(128).astype(np.float32) * 0.1
    b2 = rng.standard_normal(128).astype(np.float32) * 0.1
    bout = rng.standard_normal(16).astype(np.float32) * 0.1

    def gcn(xx, W, b):
        deg = np.bincount(ei[1], weights=ew, minlength=n) + 1.0
        dinv = 1.0 / np.sqrt(deg)
        xw = xx @ W
        msg = xw[ei[0]] * (dinv[ei[0]] * ew * dinv[ei[1]])[:, None]
        out = np.zeros_like(xw)
        np.add.at(out, ei[1], msg)
        out += xw * (dinv**2)[:, None]
        return np.maximum(out + b, 0.0)

    h = gcn(x, W1, b1)
    h = gcn(h, W2, b2)
    ref = h @ Wout.T + bout

    cfg = Cfg(n, 768, G=4)
    got = run(cfg, x, ei, ew, W1, b1, W2, b2, Wout, bout)
    err = np.abs(got - ref).max() / (np.abs(ref).max() + 1e-9)
    l2 = np.linalg.norm(got - ref) / np.linalg.norm(ref)
    print(f"SMOKE: max rel err {err:.3e}   l2 rel {l2:.3e}")
